# revision 52
# baseline (speedup 1.0000x reference)
"""Trainium2 Bass kernel for nn_Awareness_5540507812461 (online kNN "Awareness" scan).

Algorithm recap (reference.py): a strictly sequential scan over B=4096 samples.
Step i computes distances from x_i to the current reference set, inserts x_i as
a new reference iff min-dist > R (R evolves from running min/max of distances),
and predicts the label of the nearest reference after insertion.

Restructuring: if every step up to i inserted, the reference set at step i is
exactly {x_0..x_{i-1}}, so the per-step min/max distances are prefix extrema
over row i of the pairwise-distance matrix.  The device computes, per row i,
ell2[i] = min_{j<i} s_ij with s_ij = n_j - 2 x_i.x_j (so d^2 = n_i + s), plus
one GLOBAL max of s over all computed tiles (a superset of all j<i pairs; a
superset max only raises the evolving radius R, keeping verification sound).
The host adds n_i, replays the O(B) scalar recurrence with two-sided error
bounds, and verifies the all-insert speculation; margins on this workload are
~6 vs slack ~2.  On verification success each sample predicts its own label;
otherwise an exact sequential host fallback replays reference semantics.

Device scheme ("GM"):
- Distances via fp8(e4m3) DoubleRow matmuls only: the norm row n_j rides
  INSIDE the fp8 operands.  Dims 0..1019 carry x; rows 1020-1022 carry n_j
  split across 3 fp8 values at scales 32/2/1 (residual <= 0.0625); row 1023 is
  zero.  The lhsT carries -2x and the constants (32,2,1).  The dropped last-4
  data dims are bounded on host ((a4_i + prefix-max a4_j)^2, one-sided).
- Rows are interleaved across cores (core c owns rows i == c mod 8); each core
  has 4 row-stripes of 128; stripe k needs column big-tiles bt=0..k of 1024
  columns; bt==k is the diagonal "pair" tile, masked with a bf16 +-inf
  staircase via one tensor_tensor(max) (excluded elements -> +inf).
- PE: 40 DoubleRow matmuls of [K=256]x[1024 cols], c4-outer within 6 chunks of
  <=2 big-tiles so stationary weights reload only 24x; PSUM holds 2 chunks
  (2 x [128,2,1024] = all 8 banks); warmup matmuls run while inputs DMA in.
- ACT drains PSUM -> SBUF bf16 stage tiles (6 chunk-batched copies).
- DVE: staircase masks (tt-max vs +-inf), per-stripe tt-min trees, a chained
  tt-max global-max accumulator (ping-pong, no in-place ops), then halving
  tt-mins + one small reduce per output group (TensorReduce has no 2x mode,
  tensor_tensor does: all-bf16 operands run at 2 elem/cycle/lane).
"""

import os
import sys

import numpy as np

B = 4096
D = 1024
DDEV = 1020  # dims computed on device; last 4 carry the norm-split rows
NCORES = 8
NSTRIPE = 4
F32INF = np.float32(np.inf)
WARMUP_MM = 6
MASK_NEG = -3.0e38  # "pass" value for the staircase max-mask (active cols)

# chunks: (stripe k, [big-tile indices]); <=2 big-tiles each, c4-outer inside.
# Ordered so the lo column half's readers (D,C,A,B) finish mid-rep -- the next
# rep's h0 DMAs then overlap this rep's h1 chunks (F,E) instead of stalling
# the next rep's matmuls.  The final chunk (E) is a single tile to keep the
# serial ACT+DVE tail short.
CHUNKS = [
    (3, [0, 1]),
    (2, [0, 1]),
    (1, [0, 1]),
    (0, [0]),
    (3, [2, 3]),
    (2, [2]),
]
# stage slot offsets: stripe k's big-tile t lives at slot STAGE_OFF[k] + t
STAGE_OFF = {0: 0, 1: 1, 2: 3, 3: 6}

_cached = {}


def _build_bass(reps=1):
    """Build (once per `reps`) the SPMD Bass program run on all 8 cores."""
    no_gmax = os.environ.get("AWARE_NO_GMAX") == "1"
    if ("nc", reps, no_gmax) in _cached:
        return _cached[("nc", reps, no_gmax)]
    sys.path.insert(0, "/opt/trn_rl_repo")
    import concourse.bass as bass
    import concourse.mybir as mybir
    from concourse.tile import TileContext

    nc = bass.Bass(trn_type="TRN2")
    f32 = mybir.dt.float32
    bf16 = mybir.dt.bfloat16
    f8 = mybir.dt.float8e4

    rhs_d = nc.dram_tensor("rhs", [128, 4, 2, B], f8, kind="ExternalInput")
    lhs_d = nc.dram_tensor("lhsT", [128, 4, 2, 512], f8, kind="ExternalInput")
    mask_d = nc.dram_tensor("mask", [128, 1024], bf16, kind="ExternalInput")
    mm_d = nc.dram_tensor("mm", [128, 6], f32, kind="ExternalOutput")

    with TileContext(nc) as tc:
        with (
            tc.tile_pool(name="const", bufs=1) as cpool,
            tc.tile_pool(name="scratch", bufs=2) as spool,
            tc.tile_pool(name="psum", bufs=4, space="PSUM") as ppool,
        ):
            # ---- PE warmup while input DMAs stream ----
            dummy = cpool.tile([128, 512], bf16, tag="dummy")
            nc.vector.memset(dummy[:], 0.0)
            wps = ppool.tile([128, 1024], f32, tag="psum")
            for w in range(WARMUP_MM):
                nc.tensor.matmul(
                    wps[:, 0:512], lhsT=dummy[:, 0:128], rhs=dummy[:],
                    start=(w == 0), stop=(w == WARMUP_MM - 1),
                )

            # ---- persistent tiles ----
            # lo column half is persistent (its readers free it mid-rep); the
            # hi half ping-pongs across reps (its readers are the rep's last
            # chunks, so a single buffer would stall the next rep's refill)
            rhs_lo = cpool.tile([128, 4, 2, 2048], f8, tag="rhslo")
            mask_t = cpool.tile([128, 1024], bf16, tag="mask")
            stage = cpool.tile([128, 10, 1024], bf16, tag="stage")
            maskout = cpool.tile([128, 4, 1024], bf16, tag="maskout")
            tree = cpool.tile([128, 3, 1024], bf16, tag="tree")
            roots = cpool.tile([128, 4, 1024], bf16, tag="roots")
            acc = cpool.tile([128, 4, 1024], bf16, tag="acc")
            res = cpool.tile([128, 8], f32, tag="res")
            nc.vector.memset(res[:], 0.0)

            MIN = mybir.AluOpType.min
            MAX = mybir.AluOpType.max

            for _rep in range(reps):
                # ---- input DMAs in consumption order (lo half first) ----
                # lhs ping-pongs across reps (spool bufs=2) so the next rep's
                # lhs DMA never waits on this rep's readers
                # SP issues dma_starts IN ORDER, so WAR-free transfers (lhs,
                # rhs_hi ping-pong) go first; rhs_lo's WAR wait (prev rep's
                # lo readers) and the mask's (prev rep's tail mask ops) would
                # otherwise block the queue head and delay the hi half
                lhs_t = spool.tile([128, 4, 2, 512], f8, tag="lhs")
                rhs_hi = spool.tile([128, 4, 2, 2048], f8, tag="rhshi")
                nc.sync.dma_start(lhs_t[:], lhs_d[:])
                for c4 in range(4):
                    nc.sync.dma_start(
                        rhs_hi[:, c4, :, 0:2048], rhs_d[:, c4, :, 2048:4096])
                for c4 in range(4):
                    nc.sync.dma_start(
                        rhs_lo[:, c4, :, 0:2048], rhs_d[:, c4, :, 0:2048])
                nc.sync.dma_start(mask_t[:], mask_d[:])

                def run_chunk(k, bts):
                    """MMs (c4-outer) + per-tile ACT drain into stage slots."""
                    psums = []
                    for _t in range(len(bts)):
                        psum = ppool.tile([128, 1024], f32, tag="psum")
                        psums.append(psum)
                    for c4 in range(4):
                        for t, bt in enumerate(bts):
                            for h in range(2):
                                half = rhs_lo if bt < 2 else rhs_hi
                                c0 = (bt % 2) * 1024 + h * 512
                                nc.tensor.matmul(
                                    psums[t][:, h * 512:(h + 1) * 512],
                                    lhsT=lhs_t[:, c4, :, k * 128:(k + 1) * 128],
                                    rhs=half[:, c4, :, c0:c0 + 512],
                                    perf_mode=mybir.MatmulPerfMode.DoubleRow,
                                    start=(c4 == 0), stop=(c4 == 3),
                                    skip_group_check=True,
                                )
                    for t, bt in enumerate(bts):
                        nc.scalar.copy(
                            stage[:, STAGE_OFF[k] + bt, :], psums[t][:])

                def sl(k, bt):
                    return stage[:, STAGE_OFF[k] + bt, :]

                # gmax: [128,2,1024] links over the 2-tile chunks' adjacent
                # stage slices (D+C init, then A, then F), folded with the
                # single-tile chunks (B, E) at the end -- fewer, wider DVE ops
                # chunk (3,[0,1]): treeA
                run_chunk(3, [0, 1])
                nc.vector.tensor_tensor(
                    out=tree[:, 0, :], in0=sl(3, 0), in1=sl(3, 1), op=MIN)
                # chunk (2,[0,1]): treeC; gmax init link (D's and C's pairs)
                run_chunk(2, [0, 1])
                nc.vector.tensor_tensor(
                    out=tree[:, 2, :], in0=sl(2, 0), in1=sl(2, 1), op=MIN)
                if not no_gmax:
                    nc.vector.tensor_tensor(
                        out=acc[:, 0:2, :], in0=stage[:, 6:8, :],
                        in1=stage[:, 3:5, :], op=MAX)
                # chunk (1,[0,1]): mask bt1, root1; gmax link
                run_chunk(1, [0, 1])
                nc.vector.tensor_tensor(
                    out=maskout[:, 1, :], in0=sl(1, 1), in1=mask_t[:], op=MAX)
                nc.vector.tensor_tensor(
                    out=roots[:, 1, :], in0=sl(1, 0), in1=maskout[:, 1, :],
                    op=MIN)
                if not no_gmax:
                    nc.vector.tensor_tensor(
                        out=acc[:, 2:4, :], in0=acc[:, 0:2, :],
                        in1=stage[:, 1:3, :], op=MAX)
                # chunk (0,[0]): mask -> root0 (gmax of B's tile deferred)
                run_chunk(0, [0])
                nc.vector.tensor_tensor(
                    out=roots[:, 0, :], in0=sl(0, 0), in1=mask_t[:], op=MAX)
                # chunk (3,[2,3]): mask bt3, treeB, root3; gmax link
                run_chunk(3, [2, 3])
                nc.vector.tensor_tensor(
                    out=maskout[:, 3, :], in0=sl(3, 3), in1=mask_t[:], op=MAX)
                nc.vector.tensor_tensor(
                    out=tree[:, 1, :], in0=sl(3, 2), in1=maskout[:, 3, :],
                    op=MIN)
                nc.vector.tensor_tensor(
                    out=roots[:, 3, :], in0=tree[:, 0, :], in1=tree[:, 1, :],
                    op=MIN)
                if not no_gmax:
                    nc.vector.tensor_tensor(
                        out=acc[:, 0:2, :], in0=acc[:, 2:4, :],
                        in1=stage[:, 8:10, :], op=MAX)
                    g0 = spool.tile([128, 1024], bf16, tag="g0")
                    nc.vector.tensor_tensor(
                        out=g0[:], in0=acc[:, 0, :], in1=acc[:, 1, :], op=MAX)
                    g1 = spool.tile([128, 1024], bf16, tag="g1")
                    nc.vector.tensor_tensor(
                        out=g1[:], in0=g0[:], in1=sl(0, 0), op=MAX)
                # chunk (2,[2]) last: mask, root2; fold E's tile + reduce chain
                run_chunk(2, [2])
                nc.vector.tensor_tensor(
                    out=maskout[:, 2, :], in0=sl(2, 2), in1=mask_t[:], op=MAX)
                nc.vector.tensor_tensor(
                    out=roots[:, 2, :], in0=tree[:, 2, :], in1=maskout[:, 2, :],
                    op=MIN)
                if not no_gmax:
                    g2 = spool.tile([128, 1024], bf16, tag="g2")
                    nc.vector.tensor_tensor(
                        out=g2[:], in0=g1[:], in1=sl(2, 2), op=MAX)
                    nc.vector.tensor_reduce(
                        res[:, 4:5], g2[:],
                        axis=mybir.AxisListType.X, op=MAX)
                # all four stripe roots -> res[:,0:4]: one batched tt-min
                # halving (2x bf16 mode) then a half-size reduce
                rh = spool.tile([128, 4, 512], bf16, tag="rh")
                nc.vector.tensor_tensor(
                    out=rh[:], in0=roots[:, :, 0:512], in1=roots[:, :, 512:1024],
                    op=MIN)
                nc.vector.tensor_reduce(
                    res[:, 0:4], rh[:],
                    axis=mybir.AxisListType.X, op=MIN)

                nc.sync.dma_start(mm_d[:], res[:, 0:6])

    if os.environ.get("AWARE_NO_LDW_DEDUP") != "1":
        _dedup_ldweights(nc, mybir)
    _split_excess_waits(nc, mybir)
    _cached[("nc", reps, no_gmax)] = nc
    return nc


def _dedup_ldweights(nc, mybir):
    """Drop Ldweights whose stationary operand is identical to the previous
    Ldweights on the PE stream (weights persist in the array between matmuls).
    Only drops instructions with no sync waits; any on_update is migrated to
    the previous PE instruction."""
    def key(ins):
        ap = ins.ins[0]
        return (getattr(ap, "memref", None), getattr(ap, "offset", None),
                str(getattr(ap, "ap", None)), str(getattr(ap, "dtype", None)),
                str(getattr(ins, "perf_mode", None)),
                str(getattr(ins, "is_transpose", None)))

    n_drop = 0
    for fn in nc.m.functions:
        for b in fn.blocks:
            insts = b.instructions
            prev_key = [None]
            keep = []
            for ins in insts:
                op = str(ins.opcode)
                if getattr(ins, "engine", None) != mybir.EngineType.PE:
                    keep.append(ins)
                    continue
                if op == "Ldweights":
                    si = getattr(ins, "sync_info", None)
                    waits = list(si.on_wait) if si is not None and si.on_wait else []
                    upds = list(si.on_update) if si is not None and si.on_update else []
                    if key(ins) == prev_key[0] and not waits and not upds:
                        n_drop += 1
                        continue
                    prev_key[0] = key(ins)
                    keep.append(ins)
                else:
                    if op not in ("Matmult",):
                        prev_key[0] = None  # unknown PE op may clobber array
                    keep.append(ins)
            b.instructions = keep
    return n_drop


def _split_excess_waits(nc, mybir, ctrl_limit=1, other_limit=1):
    """This container's walrus build rejects >1 sync wait per instruction;
    hoist excess waits onto chained NoOps inserted before."""
    ctrl = {"Drain", "Nop", "NoOp"}
    n_split = 0
    for fn in nc.m.functions:
        for b in fn.blocks:
            insts = b.instructions
            i = 0
            while i < len(insts):
                ins = insts[i]
                limit = ctrl_limit if str(ins.opcode) in ctrl else other_limit
                si = getattr(ins, "sync_info", None)
                ow = list(si.on_wait) if si is not None and si.on_wait else []
                if len(ow) > limit:
                    si.on_wait = ow[:limit]
                    ins.sync_info = si
                    rest = ow[limit:]
                    pre = []
                    for j in range(0, len(rest), ctrl_limit):
                        n_split += 1
                        d = mybir.InstNoOp(name=f"I-wsplit-{n_split}")
                        d.engine = ins.engine
                        d.sync_info = mybir.SyncInfo(
                            on_wait=rest[j : j + ctrl_limit], on_update=[]
                        )
                        pre.append(d)
                    for j, d in enumerate(pre):
                        insts.insert(i + j, d)
                    i += len(pre)
                i += 1
    return n_split


def _prepare_inputs(xs):
    """Host-side layout prep.  Returns (in_maps, host), where host carries the
    per-row quantities for the recurrence bounds:
      n1020: ||x~_i||^2 over dims 0..1019 (fp64->fp32)
      eps_max: max_i ||x~_i - x_i|| over all 1024 dims (fp8 quantization)
      r3_max: max norm-split residual (scales 32/2/1)
      a4: ||x~_i|| over dims 1020..1023 (dropped on device, bounded on host)
    """
    import ml_dtypes

    bf = ml_dtypes.bfloat16
    f8 = ml_dtypes.float8_e4m3
    x8 = xs.astype(f8)
    xf = x8.astype(np.float32)
    eps_max = float(np.sqrt(((xf - xs) ** 2).sum(1)).max())

    n1020 = np.einsum(
        "ij,ij->i", xf[:, :DDEV].astype(np.float64), xf[:, :DDEV].astype(np.float64)
    ).astype(np.float32)
    v1 = (n1020 / 32).astype(f8).astype(np.float32)
    r1 = n1020 - 32 * v1
    v2 = (r1 / 2).astype(f8).astype(np.float32)
    r2 = r1 - 2 * v2
    v3 = r2.astype(f8).astype(np.float32)
    r3_max = float(np.abs(r2 - v3).max())
    a4 = np.sqrt((xf[:, DDEV:] ** 2).sum(1)).astype(np.float32)
    bnorm = np.sqrt((xs.astype(np.float64) ** 2).sum(1))  # true ||x_i||

    # rhs: Xhat [1024, B] fp8: rows 0..1019 = x~^T, 1020-1022 = norm split, 1023 = 0
    Xhat = np.zeros((D, B), f8)
    Xhat[:DDEV] = x8.T[:DDEV]
    Xhat[DDEV + 0] = v1.astype(f8)
    Xhat[DDEV + 1] = v2.astype(f8)
    Xhat[DDEV + 2] = v3.astype(f8)
    rhs = np.ascontiguousarray(
        Xhat.reshape(4, 2, 128, B).transpose(2, 0, 1, 3))  # [128,4,2,B]

    # lhsT: XhatL [1024, B]: rows 0..1019 = -2 x~^T (exact), consts 32/2/1, 0
    XhatL = np.zeros((D, B), np.float32)
    XhatL[:DDEV] = xf.T[:DDEV] * -2.0
    XhatL[DDEV + 0] = 32.0
    XhatL[DDEV + 1] = 2.0
    XhatL[DDEV + 2] = 1.0
    XhatL8 = XhatL.astype(f8)
    lhs_dr = XhatL8.reshape(4, 2, 128, B).transpose(2, 0, 1, 3)  # [128,4,2,B]

    p = np.arange(128)[:, None]
    q = np.arange(1024)[None, :]
    in_maps = []
    for c in range(NCORES):
        lhs_c = np.ascontiguousarray(lhs_dr[:, :, :, c::8])
        mask_c = np.where(q < 8 * p + c, np.float32(MASK_NEG),
                          np.float32(np.inf)).astype(bf)
        in_maps.append({"rhs": rhs, "lhsT": lhs_c, "mask": mask_c})
    host = {"n1020": n1020, "eps_max": eps_max, "r3_max": r3_max, "a4": a4,
            "bnorm": bnorm}
    return in_maps, host


def _reassemble(results):
    """Gather per-core [128,6] outputs -> (ell2s [B] (min_j s, no n_i), gmax_s)."""
    mins = np.empty(B, np.float32)
    gmax_s = -np.inf
    for c, r in enumerate(results):
        mm = np.asarray(r["mm"], np.float32)  # [128, 6]
        for k in range(NSTRIPE):
            rows = 1024 * k + 8 * np.arange(128) + c
            mins[rows] = mm[:, k]
        gmax_s = max(gmax_s, float(mm[:, 4:6].max()))
    return mins, gmax_s


def _scan_and_verify(ell2, gmax_s, host):
    """Replay the scalar recurrence with sound two-sided bounds; return
    (all-insert verified for TRUE dynamics, min margin)."""
    n1020 = host["n1020"].astype(np.float64)
    eps2 = 2.0 * host["eps_max"]
    # s-scale slop: norm split + bf16 tile rounding (ulp<=8 at |s|<2048) + accum
    delta_s = host["r3_max"] + 8.0 + 0.05
    a4 = host["a4"].astype(np.float64)
    pm_a4 = np.maximum.accumulate(a4)
    d4max = np.zeros(B)
    d4max[1:] = (a4[1:] + pm_a4[:-1]) ** 2

    e2 = ell2.astype(np.float64) + n1020  # device ell^2 (+n_i), j<i min
    m_lo = np.sqrt(np.maximum(e2 - delta_s, 0.0)) - eps2
    m_hi = np.sqrt(np.maximum(e2 + delta_s + d4max, 0.0)) + eps2
    if np.isfinite(gmax_s):
        M_hi = np.sqrt(np.maximum(gmax_s + n1020 + delta_s + d4max, 0.0)) + eps2
    else:
        # no-gmax variant: triangle-inequality bound on the true (unquantized)
        # distances, max_{j<i} d_ij <= ||x_i|| + max_{j<i} ||x_j||
        b = host["bnorm"].astype(np.float64)
        pm_b = np.maximum.accumulate(b)
        M_hi = np.empty(B)
        M_hi[0] = 0.0
        M_hi[1:] = b[1:] + pm_b[:-1]

    mind_hi = np.inf
    maxd_hi = 0.0
    R_hi = 1.0
    margin = np.inf
    for i in range(1, B):
        if not np.isfinite(m_lo[i]):
            return False, -np.inf
        margin = min(margin, float(m_lo[i] - R_hi))
        if not (m_lo[i] > R_hi and m_lo[i] > 0.0):
            return False, margin
        mind_hi = min(mind_hi, m_hi[i])
        maxd_hi = max(maxd_hi, M_hi[i])
        R_hi = (mind_hi + maxd_hi) / 3.0
    return True, margin


def _fallback_exact(xs, labels):
    """Exact sequential replay of the reference semantics (host, fp32)."""
    refs = np.zeros((B, D), np.float32)
    ref_labels = np.zeros((B,), np.float32)
    labels_f = labels.astype(np.float32)
    n_refs = 0
    min_d = F32INF
    max_d = np.float32(0.0)
    R = np.float32(1.0)
    preds = np.zeros(B, np.float32)
    for i in range(B):
        xi = xs[i]
        d_all = np.sqrt(np.sum((refs[:n_refs] - xi[None, :]) ** 2, axis=-1)).astype(
            np.float32
        )
        is_first = i == 0
        min_act = d_all.min() if n_refs else F32INF
        insert = is_first or (min_act > R)
        if insert:
            refs[n_refs] = xi
            ref_labels[n_refs] = labels_f[i]
        n2 = n_refs + int(insert)
        if not is_first:
            max_act = d_all.max() if n_refs else -F32INF
            min_d = np.float32(min(min_d, min_act))
            max_d = np.float32(max(max_d, max_act))
            R = np.float32((min_d + max_d) / np.float32(3.0))
        d2 = np.sqrt(np.sum((refs[:n2] - xi[None, :]) ** 2, axis=-1)).astype(np.float32)
        preds[i] = ref_labels[int(d2.argmin())]
        n_refs = n2
    return preds


def kernel(x, labels):
    x = np.asarray(x)
    labels = np.asarray(labels)
    xs = np.ascontiguousarray(x.reshape(B, D).astype(np.float32))

    sys.path.insert(0, "/opt/trn_rl_repo")
    from concourse.bass_utils import run_bass_kernel_spmd

    nc = _build_bass()
    in_maps, host = _prepare_inputs(xs)
    res = run_bass_kernel_spmd(nc, in_maps, core_ids=list(range(NCORES)))
    ell2, gmax_s = _reassemble(res.results)
    if os.environ.get("AWARE_NO_GMAX") == "1":
        gmax_s = np.nan  # host triangle bound instead
    ok, margin = _scan_and_verify(ell2, gmax_s, host)
    if os.environ.get("AWARE_DEBUG"):
        e2 = ell2.astype(np.float64) + host["n1020"]
        with np.errstate(invalid="ignore"):
            md = np.sqrt(np.maximum(e2[1:], 0)).min()
        print(f"[kernel] all-insert verified: {ok}, min margin: {margin:.4f}, "
              f"min dist: {md:.4f}, gmax_s: {gmax_s:.1f}")
    if ok:
        return labels.astype(np.float32)
    return _fallback_exact(xs, labels)


if __name__ == "__main__":
    rng = np.random.default_rng(0)
    x = rng.standard_normal((B, 1, D)).astype(np.float32)
    labels = rng.integers(0, 100, size=(B,)).astype(np.int64)
    out = kernel(x=x, labels=labels)
    print("kernel output:", out.shape, out.dtype, out[:8])
